# revision 1
# baseline (speedup 1.0000x reference)
import numpy as np
import jax
import jax.numpy as jnp
from functools import partial
from jax.sharding import Mesh, PartitionSpec as P
from jax.experimental.shard_map import shard_map

# nn_AlphaNet: hardcoded problem shapes
B, C, H, W = 50000, 1, 9, 30
D, STRIDE = 10, 10
POOL_D, POOL_STRIDE = 3, 3
HIDDEN = 30
N_CORES = 8

_X_IX = np.repeat(np.arange(H - 1), np.arange(H - 1, 0, -1))          # [36]
_Y_IX = (np.arange(_X_IX.size) - H * _X_IX + (0.5 * _X_IX + 1) * (_X_IX + 1)).astype(np.int64)
_STEP = np.arange(0, W - D + 1, STRIDE)[:, None] + np.arange(D)        # [3,10]
P_PAIRS = int(_X_IX.shape[0])   # 36
S_WIN = int(_STEP.shape[0])     # 3


def _bn_dist(x, gamma, beta, eps=1e-5):
    # BatchNorm2d train-mode batch stats over (batch, h, w) with the batch
    # axis sharded across the mesh: all-reduce per-device sum/sumsq.
    n_local = x.shape[0] * x.shape[2] * x.shape[3]
    s = jax.lax.psum(jnp.sum(x, axis=(0, 2, 3)), 'x')
    ss = jax.lax.psum(jnp.sum(x * x, axis=(0, 2, 3)), 'x')
    n = n_local * N_CORES
    mu = s / n
    var = ss / n - mu * mu
    mu = mu[None, :, None, None]
    var = var[None, :, None, None]
    return gamma[None, :, None, None] * (x - mu) * jax.lax.rsqrt(var + eps) + beta[None, :, None, None]


def _local_forward(data, bn_gamma, bn_beta, W1, b1, W2, b2):
    # data: [b_local, C, H, W]
    Xg = data[:, :, _X_IX, :][:, :, :, _STEP]   # [b,C,P,S,D]
    Yg = data[:, :, _Y_IX, :][:, :, :, _STEP]   # [b,C,P,S,D]
    Zg = data[:, :, :, _STEP]                   # [b,C,H,S,D]

    mX = Xg.mean(-1, keepdims=True)
    mY = Yg.mean(-1, keepdims=True)
    cov = jnp.sum((Xg - mX) * (Yg - mY), axis=-1) / (D - 1)
    sX = jnp.std(Xg, axis=-1, ddof=1)
    sY = jnp.std(Yg, axis=-1, ddof=1)
    corr = cov / (sX * sY)

    mZ = Zg.mean(-1)
    sZ = jnp.std(Zg, axis=-1, ddof=1)
    decay_w = (jnp.arange(D, dtype=data.dtype) + 1.0) / (0.5 * D * (D + 1))
    decay = jnp.sum(Zg * decay_w, axis=-1)
    zscore = mZ / sZ
    ret = Zg[..., -1] / Zg[..., 0] - 1.0

    feats = []
    for conv in (cov, corr, sZ, decay, zscore, ret, mZ):
        bn0 = _bn_dist(conv, bn_gamma, bn_beta)
        # pool window == S == 3, stride 3, VALID -> full reduction over axis 3
        pmax = jnp.max(bn0, axis=3, keepdims=True)
        pavg = jnp.mean(bn0, axis=3, keepdims=True)
        pmin = jnp.min(bn0, axis=3, keepdims=True)
        feats.append(bn0.reshape(bn0.shape[0], -1))
        feats.append(_bn_dist(pmax, bn_gamma, bn_beta).reshape(bn0.shape[0], -1))
        feats.append(_bn_dist(pavg, bn_gamma, bn_beta).reshape(bn0.shape[0], -1))
        feats.append(_bn_dist(pmin, bn_gamma, bn_beta).reshape(bn0.shape[0], -1))
    h = jnp.concatenate(feats, axis=1)           # [b, 702]
    h = jax.nn.relu(h @ W1.T + b1)
    return h @ W2.T + b2                         # [b, 1]


def kernel(**inputs):
    data = np.asarray(inputs["data"], dtype=np.float32)
    bn_gamma = np.asarray(inputs["bn_gamma"], dtype=np.float32)
    bn_beta = np.asarray(inputs["bn_beta"], dtype=np.float32)
    W1 = np.asarray(inputs["W1"], dtype=np.float32)
    b1 = np.asarray(inputs["b1"], dtype=np.float32)
    W2 = np.asarray(inputs["W2"], dtype=np.float32)
    b2 = np.asarray(inputs["b2"], dtype=np.float32)

    devices = jax.devices()[:N_CORES]
    mesh = Mesh(np.array(devices), ("x",))

    fwd = shard_map(
        _local_forward,
        mesh=mesh,
        in_specs=(
            P("x", None, None, None),  # data: batch-sharded
            P(None), P(None),          # bn_gamma, bn_beta: replicated
            P(None, None), P(None),    # W1, b1
            P(None, None), P(None),    # W2, b2
        ),
        out_specs=P("x", None),
    )
    fwd = jax.jit(fwd)
    out = fwd(data, bn_gamma, bn_beta, W1, b1, W2, b2)
    return np.asarray(out, dtype=np.float32)


# revision 3
# speedup vs baseline: 85.1571x; 85.1571x over previous
import numpy as np
import jax
import jax.numpy as jnp
from functools import partial
from jax.sharding import Mesh, PartitionSpec as P
from jax.experimental.shard_map import shard_map

# nn_AlphaNet: hardcoded problem shapes
B, C, H, W = 50000, 1, 9, 30
D, STRIDE = 10, 10
POOL_D, POOL_STRIDE = 3, 3
HIDDEN = 30
N_CORES = 8

_X_IX = np.repeat(np.arange(H - 1), np.arange(H - 1, 0, -1))          # [36]
_Y_IX = (np.arange(_X_IX.size) - H * _X_IX + (0.5 * _X_IX + 1) * (_X_IX + 1)).astype(np.int64)
_STEP = np.arange(0, W - D + 1, STRIDE)[:, None] + np.arange(D)        # [3,10]
P_PAIRS = int(_X_IX.shape[0])   # 36
S_WIN = int(_STEP.shape[0])     # 3


def _bn_dist(x, gamma, beta, eps=1e-5):
    # BatchNorm2d train-mode batch stats over (batch, h, w) with the batch
    # axis sharded across the mesh: all-reduce per-device sum/sumsq.
    n_local = x.shape[0] * x.shape[2] * x.shape[3]
    s = jax.lax.psum(jnp.sum(x, axis=(0, 2, 3)), 'x')
    ss = jax.lax.psum(jnp.sum(x * x, axis=(0, 2, 3)), 'x')
    n = n_local * N_CORES
    mu = s / n
    var = ss / n - mu * mu
    mu = mu[None, :, None, None]
    var = var[None, :, None, None]
    return gamma[None, :, None, None] * (x - mu) * jax.lax.rsqrt(var + eps) + beta[None, :, None, None]


def _local_forward(data, bn_gamma, bn_beta, W1, b1, W2, b2):
    # data: [b_local, C, H, W]
    Xg = data[:, :, _X_IX, :][:, :, :, _STEP]   # [b,C,P,S,D]
    Yg = data[:, :, _Y_IX, :][:, :, :, _STEP]   # [b,C,P,S,D]
    Zg = data[:, :, :, _STEP]                   # [b,C,H,S,D]

    mX = Xg.mean(-1, keepdims=True)
    mY = Yg.mean(-1, keepdims=True)
    cov = jnp.sum((Xg - mX) * (Yg - mY), axis=-1) / (D - 1)
    sX = jnp.std(Xg, axis=-1, ddof=1)
    sY = jnp.std(Yg, axis=-1, ddof=1)
    corr = cov / (sX * sY)

    mZ = Zg.mean(-1)
    sZ = jnp.std(Zg, axis=-1, ddof=1)
    decay_w = (jnp.arange(D, dtype=data.dtype) + 1.0) / (0.5 * D * (D + 1))
    decay = jnp.sum(Zg * decay_w, axis=-1)
    zscore = mZ / sZ
    ret = Zg[..., -1] / Zg[..., 0] - 1.0

    feats = []
    for conv in (cov, corr, sZ, decay, zscore, ret, mZ):
        bn0 = _bn_dist(conv, bn_gamma, bn_beta)
        # pool window == S == 3, stride 3, VALID -> full reduction over axis 3
        pmax = jnp.max(bn0, axis=3, keepdims=True)
        pavg = jnp.mean(bn0, axis=3, keepdims=True)
        pmin = jnp.min(bn0, axis=3, keepdims=True)
        feats.append(bn0.reshape(bn0.shape[0], -1))
        feats.append(_bn_dist(pmax, bn_gamma, bn_beta).reshape(bn0.shape[0], -1))
        feats.append(_bn_dist(pavg, bn_gamma, bn_beta).reshape(bn0.shape[0], -1))
        feats.append(_bn_dist(pmin, bn_gamma, bn_beta).reshape(bn0.shape[0], -1))
    h = jnp.concatenate(feats, axis=1)           # [b, 702]
    h = jax.nn.relu(h @ W1.T + b1)
    return h @ W2.T + b2                         # [b, 1]


_FWD_CACHE = {}


def _get_fwd():
    if "fwd" not in _FWD_CACHE:
        devices = jax.devices()[:N_CORES]
        mesh = Mesh(np.array(devices), ("x",))
        fwd = shard_map(
            _local_forward,
            mesh=mesh,
            in_specs=(
                P("x", None, None, None),
                P(None), P(None),
                P(None, None), P(None),
                P(None, None), P(None),
            ),
            out_specs=P("x", None),
        )
        _FWD_CACHE["fwd"] = jax.jit(fwd)
    return _FWD_CACHE["fwd"]


def kernel(**inputs):
    data = np.asarray(inputs["data"], dtype=np.float32)
    bn_gamma = np.asarray(inputs["bn_gamma"], dtype=np.float32)
    bn_beta = np.asarray(inputs["bn_beta"], dtype=np.float32)
    W1 = np.asarray(inputs["W1"], dtype=np.float32)
    b1 = np.asarray(inputs["b1"], dtype=np.float32)
    W2 = np.asarray(inputs["W2"], dtype=np.float32)
    b2 = np.asarray(inputs["b2"], dtype=np.float32)

    fwd = _get_fwd()
    out = fwd(data, bn_gamma, bn_beta, W1, b1, W2, b2)
    out = np.asarray(out, dtype=np.float32)
    return out
